# revision 1
# baseline (speedup 1.0000x reference)
"""Code2Vec forward kernel for Trainium2 (Bass/Tile), data-parallel over batch.

Model (per batch row b):
  es = node_emb[starts[b]]; ep = path_emb[paths[b]]; ee = node_emb[ends[b]]
  x  = tanh([es|ep|ee] @ W.T)            # [T, E]
  z  = softmax(x @ a)                    # [T], over full T
  v  = sum_t x[t] * (z*mask)[t]          # [E]
  out = v @ out_W.T + out_b              # [OUT]

Sharding: 8 NeuronCores, 8 batch rows each; embedding tables replicated.
The gathers (3 * 512 rows * 512B per batch row) are the memory-bound part and
run as batched indirect DMAs. The gathered [token, d] tiles are transposed on
the PE (contraction dim must sit on partitions), packed 3-at-a-time into one
PSUM bank, then the fused matmul+tanh / attention / projection pipeline runs
on PE/ACT/DVE.
"""

import sys

import numpy as np

sys.path.insert(0, "/opt/trn_rl_repo")

B, T, E = 64, 512, 128
NODES, PATHS, OUT = 100000, 200000, 1000
PAD = 1
NCORES = 8
BC = B // NCORES          # batch rows per core
CHUNKS = T // 128         # 128-token chunks per batch row
J = BC * CHUNKS           # token tiles per core (32)
JH = J // 2               # half (low/high batch groups)

_BUILT = None
LAST_RESULTS = None
TRACE = False


def _build():
    """Build the (SPMD, identical across cores) Bass kernel once."""
    from contextlib import ExitStack

    import concourse.bacc as bacc
    import concourse.bass as bass
    import concourse.tile as tile
    from concourse import mybir

    f32 = mybir.dt.float32
    i32 = mybir.dt.int32

    nc = bacc.Bacc("TRN2", target_bir_lowering=False, debug=False, num_devices=NCORES)

    d_sidx = nc.dram_tensor("s_idx", [128, J], i32, kind="ExternalInput")
    d_pidx = nc.dram_tensor("p_idx", [128, J], i32, kind="ExternalInput")
    d_eidx = nc.dram_tensor("e_idx", [128, J], i32, kind="ExternalInput")
    d_node = nc.dram_tensor("node_emb", [NODES, E], f32, kind="ExternalInput")
    d_path = nc.dram_tensor("path_emb", [PATHS, E], f32, kind="ExternalInput")
    d_wt = nc.dram_tensor("wt", [128, 3, E], f32, kind="ExternalInput")
    d_aoh = nc.dram_tensor("a_oh", [E, BC * BC], f32, kind="ExternalInput")
    d_ohr = nc.dram_tensor("oh_rows", [128, BC * 128], f32, kind="ExternalInput")
    d_mask = nc.dram_tensor("mask", [BC, T], f32, kind="ExternalInput")
    d_owt = nc.dram_tensor("out_wt", [E, OUT], f32, kind="ExternalInput")
    d_ob = nc.dram_tensor("out_b", [BC, OUT], f32, kind="ExternalInput")
    d_ident = nc.dram_tensor("ident", [128, 128], f32, kind="ExternalInput")
    d_out = nc.dram_tensor("out", [BC, OUT], f32, kind="ExternalOutput")

    with ExitStack() as ctx:
        tc = ctx.enter_context(tile.TileContext(nc))
        const = ctx.enter_context(tc.tile_pool(name="const", bufs=1))
        gath = ctx.enter_context(tc.tile_pool(name="gath", bufs=1))
        ctp = ctx.enter_context(tc.tile_pool(name="ct", bufs=2))
        xtp = ctx.enter_context(tc.tile_pool(name="xt", bufs=BC))
        scrp = ctx.enter_context(tc.tile_pool(name="scr", bufs=2))
        smallp = ctx.enter_context(tc.tile_pool(name="small", bufs=1))
        p_tr = ctx.enter_context(tc.tile_pool(name="ptr", bufs=3, space="PSUM"))
        p_x = ctx.enter_context(tc.tile_pool(name="px", bufs=2, space="PSUM"))
        p_s = ctx.enter_context(tc.tile_pool(name="ps", bufs=1, space="PSUM"))

        # ---- constants / small inputs ----
        ident = const.tile([128, 128], f32)
        nc.sync.dma_start(out=ident[:], in_=d_ident[:])

        wt_sb = const.tile([128, 3, E], f32)
        nc.sync.dma_start(out=wt_sb[:], in_=d_wt[:])
        aoh_sb = const.tile([E, BC * BC], f32)
        nc.sync.dma_start(out=aoh_sb[:], in_=d_aoh[:])
        ohr_sb = const.tile([128, BC * 128], f32)
        nc.sync.dma_start(out=ohr_sb[:], in_=d_ohr[:])
        mask_sb = const.tile([BC, T], f32)
        nc.sync.dma_start(out=mask_sb[:], in_=d_mask[:])
        owt_sb = const.tile([E, OUT], f32)
        nc.sync.dma_start(out=owt_sb[:], in_=d_owt[:])
        ob_sb = const.tile([BC, OUT], f32)
        nc.sync.dma_start(out=ob_sb[:], in_=d_ob[:])

        sidx_sb = const.tile([128, J], i32)
        nc.sync.dma_start(out=sidx_sb[:], in_=d_sidx[:])
        pidx_sb = const.tile([128, J], i32)
        nc.sync.dma_start(out=pidx_sb[:], in_=d_pidx[:])
        eidx_sb = const.tile([128, J], i32)
        nc.sync.dma_start(out=eidx_sb[:], in_=d_eidx[:])

        # ---- gathers: [P,1]-offset indirect DMAs (128 rows each), b-major ----
        # g_*[p, j, :] = table[idx[p, j], :]
        g_es = gath.tile([128, J, E], f32)
        g_ep = gath.tile([128, J, E], f32)
        g_ee = gath.tile([128, J, E], f32)
        for j in range(J):
            for g, idx, table in (
                (g_es, sidx_sb, d_node),
                (g_ep, pidx_sb, d_path),
                (g_ee, eidx_sb, d_node),
            ):
                nc.gpsimd.indirect_dma_start(
                    out=g[:, j, :],
                    out_offset=None,
                    in_=table[:],
                    in_offset=bass.IndirectOffsetOnAxis(ap=idx[:, j:j + 1], axis=0),
                )

        # ---- per-batch-row pipeline ----
        S_ps = p_s.tile([BC, T], f32, tag="s")
        xt_tiles = []
        for b in range(BC):
            jbase = CHUNKS * b
            # transpose gathered [t, d] chunks -> cT[d, table, t]
            ct = ctp.tile([128, 3, T], f32, tag="ct")
            for c in range(CHUNKS):
                tr = p_tr.tile([128, 3, 128], f32, tag="tr")
                for k, g in enumerate((g_es, g_ep, g_ee)):
                    nc.tensor.transpose(
                        out=tr[:, k, :],
                        in_=g[:, jbase + c, :],
                        identity=ident[:],
                    )
                nc.vector.tensor_copy(
                    out=ct[:, :, c * 128:(c + 1) * 128], in_=tr[:]
                )
            # x^T[e, t] = sum_k wt[:,k,:].T @ cT[:,k,:]
            px = p_x.tile([128, T], f32, tag="x")
            for k in range(3):
                nc.tensor.matmul(
                    out=px[:],
                    lhsT=wt_sb[:, k, :],
                    rhs=ct[:, k, :],
                    start=(k == 0),
                    stop=(k == 2),
                )
            xt = xtp.tile([128, T], f32, tag="xt")
            nc.scalar.activation(
                out=xt[:], in_=px[:], func=mybir.ActivationFunctionType.Tanh
            )
            xt_tiles.append(xt)
            # scores: S[b, t] = a . x^T[:, t]   (one-hot col-b lhsT accumulation)
            nc.tensor.matmul(
                out=S_ps[:],
                lhsT=aoh_sb[:, b * BC:(b + 1) * BC],
                rhs=xt[:],
                start=(b == 0),
                stop=(b == BC - 1),
            )

        # ---- masked softmax over t (free dim), [BC, T] ----
        negmax = smallp.tile([BC, 1], f32, tag="negmax")
        nc.vector.tensor_reduce(
            out=negmax[:], in_=S_ps[:], axis=mybir.AxisListType.X,
            op=mybir.AluOpType.max, negate=True,
        )
        ex = smallp.tile([BC, T], f32, tag="ex")
        nc.scalar.activation(
            out=ex[:], in_=S_ps[:], func=mybir.ActivationFunctionType.Exp,
            bias=negmax[:], scale=1.0,
        )
        ssum = smallp.tile([BC, 1], f32, tag="ssum")
        nc.vector.tensor_reduce(
            out=ssum[:], in_=ex[:], axis=mybir.AxisListType.X,
            op=mybir.AluOpType.add,
        )
        rec = smallp.tile([BC, 1], f32, tag="rec")
        nc.vector.reciprocal(out=rec[:], in_=ssum[:])
        wm = smallp.tile([BC, T], f32, tag="wm")
        nc.vector.tensor_tensor(
            out=wm[:], in0=ex[:], in1=mask_sb[:], op=mybir.AluOpType.mult
        )
        wfin = smallp.tile([BC, T], f32, tag="wfin")
        nc.vector.tensor_scalar(
            out=wfin[:], in0=wm[:], scalar1=rec[:], scalar2=None,
            op0=mybir.AluOpType.mult,
        )

        # ---- v^T[e, b] = sum_t x^T[e, t] * w[b, t] ----
        # broadcast w rows across partitions via K=128 one-hot-row matmul
        # (K=8 matmuls hang the PE; engines can't partition-broadcast), then
        # fused multiply+reduce on DVE
        wfp = smallp.tile([128, T], f32, tag="wfp")
        nc.vector.memset(wfp[:], 0.0)
        nc.vector.tensor_copy(out=wfp[0:BC, :], in_=wfin[:])
        vt_sb = smallp.tile([128, BC], f32, tag="vt")
        for b in range(BC):
            wb = p_x.tile([128, T], f32, tag="x")  # reuse x psum slots
            nc.tensor.matmul(
                out=wb[:],
                lhsT=ohr_sb[:, b * 128:(b + 1) * 128],
                rhs=wfp[:],
                start=True,
                stop=True,
            )
            scr = scrp.tile([128, T], f32, tag="scr")
            nc.vector.tensor_tensor(
                out=scr[:], in0=xt_tiles[b][:], in1=wb[:],
                op=mybir.AluOpType.mult,
            )
            nc.vector.tensor_reduce(
                out=vt_sb[:, b:b + 1], in_=scr[:],
                axis=mybir.AxisListType.X, op=mybir.AluOpType.add,
            )

        # ---- out = v @ out_W.T + out_b ----  (one PSUM bank per matmul)
        o_sb = smallp.tile([BC, OUT], f32, tag="o")
        po_a = p_s.tile([BC, 512], f32, tag="poa")
        nc.tensor.matmul(
            out=po_a[:], lhsT=vt_sb[:], rhs=owt_sb[:, 0:512],
            start=True, stop=True,
        )
        nc.vector.tensor_tensor(
            out=o_sb[:, 0:512], in0=po_a[:], in1=ob_sb[:, 0:512],
            op=mybir.AluOpType.add,
        )
        po_b = p_s.tile([BC, OUT - 512], f32, tag="pob")
        nc.tensor.matmul(
            out=po_b[:], lhsT=vt_sb[:], rhs=owt_sb[:, 512:OUT],
            start=True, stop=True,
        )
        nc.vector.tensor_tensor(
            out=o_sb[:, 512:OUT], in0=po_b[:], in1=ob_sb[:, 512:OUT],
            op=mybir.AluOpType.add,
        )
        nc.sync.dma_start(out=d_out[:], in_=o_sb[:])

    nc.compile()
    return nc


def _get_built():
    global _BUILT
    if _BUILT is None:
        _BUILT = _build()
    return _BUILT


def _prep_shared(node_emb, path_emb, W, a, out_W, out_b):
    node_z = np.array(node_emb, dtype=np.float32, copy=True)
    node_z[PAD, :] = 0.0
    path_z = np.ascontiguousarray(path_emb, dtype=np.float32)
    # wt[d, k, e] = W[e, 128k + d]
    wt = np.ascontiguousarray(
        np.asarray(W, dtype=np.float32).reshape(E, 3, E).transpose(2, 1, 0)
    )
    a_oh = np.zeros((E, BC * BC), dtype=np.float32)
    for b in range(BC):
        a_oh[:, b * BC + b] = np.asarray(a, dtype=np.float32)
    oh_rows = np.zeros((128, BC * 128), dtype=np.float32)
    for b in range(BC):
        oh_rows[b, b * 128:(b + 1) * 128] = 1.0
    owt = np.ascontiguousarray(np.asarray(out_W, dtype=np.float32).T)
    ob = np.ascontiguousarray(
        np.broadcast_to(np.asarray(out_b, dtype=np.float32), (BC, OUT))
    )
    return node_z, path_z, wt, a_oh, oh_rows, owt, ob


def _idx_tile(idx_rows):
    # [BC, T] -> [128, J] with tile[p, 4b+c] = idx_rows[b, 128c + p]
    return np.ascontiguousarray(
        np.asarray(idx_rows).reshape(BC, CHUNKS, 128).transpose(2, 0, 1)
        .reshape(128, J).astype(np.int32)
    )


def make_in_maps(starts, paths, ends, length, node_emb, path_emb, W, a, out_W, out_b):
    node_z, path_z, wt, a_oh, oh_rows, owt, ob = _prep_shared(
        node_emb, path_emb, W, a, out_W, out_b
    )
    length = np.asarray(length)
    in_maps = []
    for k in range(NCORES):
        rows = slice(k * BC, (k + 1) * BC)
        mask = (
            np.arange(T)[None, :] < np.asarray(length[rows])[:, None]
        ).astype(np.float32)
        in_maps.append(dict(
            s_idx=_idx_tile(starts[rows]),
            p_idx=_idx_tile(paths[rows]),
            e_idx=_idx_tile(ends[rows]),
            node_emb=node_z,
            path_emb=path_z,
            wt=wt,
            a_oh=a_oh,
            oh_rows=oh_rows,
            mask=np.ascontiguousarray(mask),
            out_wt=owt,
            out_b=ob,
            ident=np.eye(128, dtype=np.float32),
        ))
    return in_maps


def kernel(starts, paths, ends, length, node_emb, path_emb, W, a, out_W, out_b):
    global LAST_RESULTS
    import os

    if not TRACE:
        # trace=True needs antenv.axon_hooks, absent on this image; make sure
        # an ambient BASS_TRACE can't route us into that path
        os.environ["BASS_NEVER_TRACE"] = "1"
    from concourse.bass_utils import run_bass_kernel_spmd

    nc = _get_built()
    in_maps = make_in_maps(
        starts, paths, ends, length, node_emb, path_emb, W, a, out_W, out_b
    )
    res = run_bass_kernel_spmd(
        nc, in_maps, core_ids=list(range(NCORES)), trace=TRACE
    )
    LAST_RESULTS = res
    return np.concatenate([r["out"] for r in res.results], axis=0)

